# revision 62
# baseline (speedup 1.0000x reference)
"""Trainium2 Bass kernel for nn_AffinityPropagate2 (8-iteration dual-dilation
affinity propagation with per-pixel softmax kernels).

Contract: kernel(**inputs) takes FULL numpy inputs
    guided1 [4,9,352,1216] f32, guided2 [4,9,352,1216] f32,
    fuse    [4,2,352,1216] f32, x [4,1,352,1216] f32
and returns the FULL output [4,1,352,1216] f32.

Strategy (8 NeuronCores, SPMD, no cross-core communication):
  - Shard: core c = (batch b = c//2, H-half = c%2). Each core owns 176 output
    rows plus a one-sided ghost zone (interior boundary) that shrinks 2 rows
    per iteration; the outer boundary is the true image edge (zero padding).
    Half-1 shards are row-flipped on the host (and the 3x3 tap planes row-
    mirrored) so a single SPMD program serves all 8 cores.
  - On-chip layout: W padded 1216->1280 and split into 128 column strips of
    10 (partition dim = strip). H lives in the free dim, so all stencil
    shifts are free-dim offsets. Each strip carries 2 halo columns per side;
    halos are refreshed each iteration with a tiny TensorE permutation
    matmul (cross-partition shift) + ScalarE PSUM->SBUF copy.
  - Softmax is folded once into 17 per-tap weight planes:
        w1_k = exp(g1_k) * f1 / sum_j exp(g1_j)   (dil 1)
        w2_k = exp(g2_k) * f2 / sum_j exp(g2_j)   (dil 2)
    with the two center taps merged. Zero-padding of the image is emulated
    by zeros in `fuse` pad columns (=> zero weights => pad stays zero).
  - Iterations: x' = sum_k w_k * shift_k(x): 17 tensor_tensor mults + 16 adds
    on VectorE per iteration. Compute dtype fp16 (DVE 2x mode); a one-column-
    shifted copy of x (maintained by ScalarE) keeps odd-column taps 4B-aligned.
"""

import numpy as np

# ---------------------------------------------------------------- geometry

def make_geom(B=4, H=352, W=1216, SW=10, NS=128, PT=8, dt_name="float16"):
    HH = H // 2
    g = dict(
        B=B, H=H, W=W, SW=SW, NS=NS, PT=PT, dt_name=dt_name,
        Wp=NS * SW,
        HH=HH,
        RW=HH + 2 * (PT - 1),      # weight rows per shard (incl. ghost)
        RXL=HH + 2 * PT,           # x rows loaded per shard
        SWH=SW + 4,                # strip width incl. 2+2 halo cols
    )
    g["RX"] = g["RXL"] + 4         # x rows incl. 2+2 zero-pad rows
    assert g["Wp"] >= W and NS <= 128
    assert g["RW"] * 2 <= 512, "halo matmul free dim must fit one PSUM bank"
    return g


# ---------------------------------------------------------------- device IR

def emit(tc, outs, ins, g):
    """Emit the SPMD per-core program into TileContext tc.

    ins: dict of DRAM APs: g1 [9,NS,RW,SW], g2 [9,NS,RW,SW], fz [2,NS,RW,SW],
         x0 [NS,RX,SWH], pl [NS,NS], pr [NS,NS]  (all in compute dtype except
         g1/g2/fz/x0 which are already compute dtype from the host)
    outs: y [NS,HH,SW] float32
    """
    import concourse.mybir as mybir

    nc = tc.nc
    NS, SW, SWH, RW, RX, HH, PT = (
        g["NS"], g["SW"], g["SWH"], g["RW"], g["RX"], g["HH"], g["PT"])
    DT = getattr(mybir.dt, g["dt_name"])
    F32 = mybir.dt.float32
    two_byte = mybir.dt.size(DT) == 2
    EXP = mybir.ActivationFunctionType.Exp

    g1, g2, fz, x0, pl, pr = (ins[k] for k in ("g1", "g2", "fz", "x0", "pl", "pr"))
    y = outs["y"]

    from contextlib import ExitStack
    ctx = tc.nc._emit_ctx = ExitStack()  # keep pools open until trace ends
    pool = ctx.enter_context(tc.tile_pool(name="main", bufs=1))
    psp = ctx.enter_context(tc.tile_pool(name="ps", bufs=2, space="PSUM"))

    w1 = pool.tile([NS, 9, RW, SW], DT, name="w1", tag="w1")
    w2 = pool.tile([NS, 9, RW, SW], DT, name="w2", tag="w2")
    fg = pool.tile([NS, 2, RW, SW], DT, name="fg", tag="fg")
    xb = [pool.tile([NS, RX, SWH], DT, name=f"xb{i}", tag=f"xb{i}") for i in range(2)]
    xo = [pool.tile([NS, RX, SWH], DT, name=f"xo{i}", tag=f"xo{i}") for i in range(2)] if two_byte else None
    acc = pool.tile([NS, RW, SW], DT, name="acc", tag="acc")
    tmp = pool.tile([NS, RW, SW], DT, name="tmp", tag="tmp")
    tm2 = pool.tile([NS, RW, SW], DT, name="tm2", tag="tm2")
    p3 = pool.tile([NS, 9, RW, SW], DT, name="p3", tag="p3")
    s32 = pool.tile([NS, RW, SW], F32, name="s32", tag="s32")
    r32 = pool.tile([NS, RW, SW], F32, name="r32", tag="r32")
    plt = pool.tile([NS, NS], DT, name="plt", tag="plt")
    prt = pool.tile([NS, NS], DT, name="prt", tag="prt")
    pit = pool.tile([NS, NS], DT, name="pit", tag="pit")
    yc = pool.tile([NS, HH, SW], DT, name="yc", tag="yc")
    # PSUM accumulator chunks: TensorE identity-matmuls sum the 17 product
    # planes here (f32), ScalarE copies the result back to fp16 SBUF
    CH = 48
    NCH = (RW + CH - 1) // CH
    assert NCH * 1 + 2 * 2 <= 8 and CH * SW <= 512
    pacc = [psp.tile([NS, CH, SW], F32, name=f"pacc{i}", tag=f"pacc{i}", bufs=1)
            for i in range(NCH)]

    # ---- loads, ordered so everything the dil1 half of iteration 0 needs
    # arrives first (and within each stencil, in tap-group consumption
    # order); the dil1 compute then overlaps the g2 stream
    nc.sync.dma_start(out=plt, in_=pl)
    nc.sync.dma_start(out=prt, in_=pr)
    nc.sync.dma_start(out=pit, in_=ins["pi"])
    nc.sync.dma_start(out=xb[0], in_=x0)
    for k in (1, 4, 7, 0, 3, 6, 2, 5, 8):   # dw=0 group, then odd-dw groups
        nc.sync.dma_start(out=w1[:, k], in_=g1[k])
    nc.sync.dma_start(out=fg[:, 0], in_=fz[0])
    nc.sync.dma_start(out=fg[:, 1], in_=fz[1])
    for k in (0, 1, 2, 6, 7, 8, 3, 5, 4):   # row groups, then dw=+-2, center
        nc.sync.dma_start(out=w2[:, k], in_=g2[k])

    # ---- x init: only the two top pad rows of the write buffers ever get
    # read without being written (global rows -2/-1 must stay zero); the
    # host-prepped x0 covers everything else
    nc.vector.memset(xb[1][:, 0:2, :], 0.0)
    if two_byte:
        nc.vector.memset(xo[1][:, 0:2, :], 0.0)
        nc.scalar.copy(out=xo[0][:, :, 0:SWH - 1], in_=xb[0][:, :, 1:SWH])

    PREP = g.get("PREP_LEVEL", 3)  # perf decomposition only
    # softmax normalizer chain for one stencil:
    # fg[s] <- f_s / sum_k exp(g_s[k])  (exp planes already in wt).
    # The 9-plane sum rides TensorE identity-matmul PSUM accumulation, so
    # the DVE only does the reciprocal and the fuse multiply.
    def norm_chain(s, wt):
        for j in range(9):
            for ci, (r0, rows) in enumerate(chunks(RW)):
                nc.tensor.matmul(
                    pacc[ci][:, 0:rows], pit, wt[:, j, r0:r0 + rows, :],
                    start=(j == 0), stop=(j == 8))
        for ci, (r0, rows) in enumerate(chunks(RW)):
            nc.scalar.copy(out=s32[:, r0:r0 + rows, :], in_=pacc[ci][:, 0:rows])
        # ~51-ULP fp32 reciprocal: far below the fp16 pipeline noise floor
        nc.vector.reciprocal_approx_fast(out=r32, in_=s32)
        nc.vector.tensor_mul(fg[:, s], fg[:, s], r32)

    import concourse.bass as bass_mod

    def tap_src(dh, dw, Rt, xin, xoin):
        if two_byte and (dw % 2 != 0):
            return xoin[:, 2 + dh:2 + dh + Rt, 1 + dw:1 + dw + SW]
        return xin[:, 2 + dh:2 + dh + Rt, 2 + dw:2 + dw + SW]

    PS = RW * SW  # weight plane stride in elements

    def with_dims(base, dims):
        """insert extra leading free dims [step, count] into a sliced AP"""
        return bass_mod.AP(tensor=base.tensor, offset=base.offset,
                           ap=[base.ap[0], *dims, *base.ap[1:]])

    def x_grp(xsrc, row0, col0, dims, Rt):
        return with_dims(xsrc[:, row0:row0 + Rt, col0:col0 + SW], dims)

    def w_grp(wt, k0, dims, Rt):
        return with_dims(wt[:, k0, 0:Rt, :], dims)

    # fused tap-group multiplies — ONE tensor_mul per 3-tap (or 2-tap) group.
    # The ISA tensor pattern is TENSOR3D: at most 3 free dims per AP, so
    # 6-tap (2-lead-dim) fusions are not encodable.
    #   A    : dil1 dw=0  (k=1,4,7)   3 products (halo-independent, first)
    #   ODD-+: dil1 dw=-1 (k=0,3,6) / dw=+1 (k=2,5,8)  (read the x-odd copy)
    #   B / C: dil2 dh=-2 (k=0,1,2) / dh=+2 (k=6,7,8)
    #   D    : dil2 dh=0 dw=+-2 (k=3,5)   2 products
    def mul_A(wt, Rt, xin, s):
        nc.vector.tensor_mul(
            p3[:, s:s + 3, 0:Rt, :],
            x_grp(xin, 1, 2, [[SWH, 3]], Rt), wt[:, 1:8:3, 0:Rt, :])

    def mul_A_split(wt, Rt, xin, s):
        # row-split so range j only reads x rows covered by the previous
        # iteration's PSUM copy-out chunks 0..j — the DVE starts on the
        # first chunk instead of waiting for all four
        bounds = [0] + [CH * j - 2 for j in range(1, (Rt + CH - 1) // CH)] + [Rt]
        for a, b in zip(bounds[:-1], bounds[1:]):
            bx = xin[:, 1 + a:1 + b, 2:2 + SW]
            nc.vector.tensor_mul(
                p3[:, s:s + 3, a:b, :], with_dims(bx, [[SWH, 3]]),
                wt[:, 1:8:3, a:b, :])

    def mul_ODD(wt, Rt, xin, xoin, s, dw):
        xsrc, c0 = (xoin, 1) if two_byte else (xin, 2)
        nc.vector.tensor_mul(
            p3[:, s:s + 3, 0:Rt, :],
            x_grp(xsrc, 1, c0 + dw, [[SWH, 3]], Rt),
            wt[:, 1 + dw:8 + dw:3, 0:Rt, :])

    def mul_B(wt, Rt, xin, s):
        nc.vector.tensor_mul(
            p3[:, s:s + 3, 0:Rt, :],
            x_grp(xin, 0, 0, [[2, 3]], Rt), wt[:, 0:3, 0:Rt, :])

    def mul_C(wt, Rt, xin, s):
        nc.vector.tensor_mul(
            p3[:, s:s + 3, 0:Rt, :],
            x_grp(xin, 4, 0, [[2, 3]], Rt), wt[:, 6:9, 0:Rt, :])

    def mul_D(wt, Rt, xin, s):
        nc.vector.tensor_mul(
            p3[:, s:s + 2, 0:Rt, :],
            x_grp(xin, 2, 0, [[4, 2]], Rt), wt[:, 3:6:2, 0:Rt, :])

    def padd(d, s, n, Rt):  # p3[d:d+n] += p3[s:s+n]
        nc.vector.tensor_add(p3[:, d:d + n, 0:Rt, :], p3[:, d:d + n, 0:Rt, :],
                             p3[:, s:s + n, 0:Rt, :])

    def chunks(Rt):
        return [(ci * CH, min(CH, Rt - ci * CH))
                for ci in range((Rt + CH - 1) // CH)]

    def mm_sum(s, n, Rt, first, last):
        # accumulate product planes p3[s:s+n] into the PSUM chunks via
        # identity matmuls (PE's native accumulate; frees the DVE adds)
        for j in range(n):
            for ci, (r0, rows) in enumerate(chunks(Rt)):
                nc.tensor.matmul(
                    pacc[ci][:, 0:rows], pit,
                    p3[:, s + j, r0:r0 + rows, :],
                    start=(first and j == 0), stop=(last and j == n - 1))

    def mm_copyout(Rt, dst_rows_of):
        # PSUM f32 sums -> fp16 SBUF destination rows via ScalarE
        for ci, (r0, rows) in enumerate(chunks(Rt)):
            nc.scalar.copy(out=dst_rows_of(r0, rows), in_=pacc[ci][:, 0:rows])

    def mm_sum_last_fused(s, n, Rt, dst_rows_of):
        # last group chunk-major with immediate per-chunk copy-out, so the
        # ScalarE copies overlap PE's remaining chunks
        for ci, (r0, rows) in enumerate(chunks(Rt)):
            for j in range(n):
                nc.tensor.matmul(
                    pacc[ci][:, 0:rows], pit, p3[:, s + j, r0:r0 + rows, :],
                    start=False, stop=(j == n - 1))
            nc.scalar.copy(out=dst_rows_of(r0, rows), in_=pacc[ci][:, 0:rows])

    def final_store(xout, Rt, a, b):
        # single add writing the new x interior; the following halo chain is
        # hidden behind the next iteration's halo-independent lead taps
        nc.vector.tensor_add(
            xout[:, 2:2 + Rt, 2:2 + SW], a[:, 0:Rt], b[:, 0:Rt])

    def halo_refresh(xout, xoout, Rt):
        # left halo cols [0:2) <- neighbor p-1 cols [SW:SW+2) via TensorE
        # permutation matmul (the only cross-partition path in the loop)
        psl = psp.tile([NS, RW, 2], F32, name="psl", tag="psl")
        nc.tensor.matmul(psl[:, 0:Rt], plt, xout[:, 2:2 + Rt, SW:SW + 2],
                         start=True, stop=True)
        nc.scalar.copy(out=xout[:, 2:2 + Rt, 0:2], in_=psl[:, 0:Rt])
        psr = psp.tile([NS, RW, 2], F32, name="psr", tag="psr")
        nc.tensor.matmul(psr[:, 0:Rt], prt, xout[:, 2:2 + Rt, 2:4],
                         start=True, stop=True)
        nc.scalar.copy(out=xout[:, 2:2 + Rt, SW + 2:SW + 4], in_=psr[:, 0:Rt])
        if two_byte:
            nc.scalar.copy(out=xoout[:, 2:2 + Rt, 0:SWH - 1],
                           in_=xout[:, 2:2 + Rt, 1:SWH])

    PT = g.get("PTE", PT)  # emit fewer iterations (perf decomposition only)

    # ---- iteration 0, interleaved with preprocessing so the whole dil1 half
    # (which only needs g1/fuse/x) overlaps the g2 DMA stream.
    # Runs on raw exp planes with per-stencil accumulators:
    #   x1 = acc1*G1 + acc2*G2   (associativity of the softmax fold)
    if PT >= 1:
        Rt = RW
        xin, xout = xb[0], xb[1]
        xoin = xo[0] if two_byte else None
        xoout = xo[1] if two_byte else None
        # g1-only phase
        for k in range(9):
            nc.scalar.activation(out=w1[:, k], in_=w1[:, k], func=EXP)
        if PREP >= 2:
            norm_chain(0, w1)
        mul_A(w1, Rt, xin, 0)
        mm_sum(0, 3, Rt, True, False)
        mul_ODD(w1, Rt, xin, xoin, 3, -1)
        mm_sum(3, 3, Rt, False, False)
        mul_ODD(w1, Rt, xin, xoin, 6, +1)
        mm_sum(6, 3, Rt, False, True)
        mm_copyout(Rt, lambda r0, rows: acc[:, r0:r0 + rows, :])
        nc.vector.tensor_mul(acc, acc, fg[:, 0])
        # g2 phase
        for k in range(9):
            nc.scalar.activation(out=w2[:, k], in_=w2[:, k], func=EXP)
        if PREP >= 2:
            norm_chain(1, w2)
        mul_B(w2, Rt, xin, 0)
        mm_sum(0, 3, Rt, True, False)
        mul_C(w2, Rt, xin, 3)
        mm_sum(3, 3, Rt, False, False)
        mul_D(w2, Rt, xin, 6)
        nc.vector.tensor_mul(p3[:, 8, 0:Rt, :], tap_src(0, 0, Rt, xin, xoin),
                             w2[:, 4, 0:Rt, :])  # dil2 center tap
        mm_sum(6, 2, Rt, False, False)
        mm_sum(8, 1, Rt, False, True)
        mm_copyout(Rt, lambda r0, rows: tm2[:, r0:r0 + rows, :])
        nc.vector.tensor_mul(tm2, tm2, fg[:, 1])
        final_store(xout, Rt, acc, tm2)
        halo_refresh(xout, xoout, Rt)
        # fold softmax normalizers into the tap planes for iterations 1+
        # (off the load critical path; WAR deps on the iter-0 reads are
        # handled by the tile scheduler). Broadcast-AP: one mult folds 4
        # planes.
        nc.vector.tensor_mul(tmp, w2[:, 4], fg[:, 1])
        nc.vector.tensor_mul(w1[:, 4], w1[:, 4], fg[:, 0])
        nc.vector.tensor_add(w1[:, 4], w1[:, 4], tmp)
        for wt, s in ((w1, 0), (w2, 1)):
            b = fg[:, s]
            g4 = bass_mod.AP(tensor=b.tensor, offset=b.offset,
                             ap=[b.ap[0], [0, 4], b.ap[1], b.ap[2]])
            nc.vector.tensor_mul(wt[:, 0:4], wt[:, 0:4], g4)
            nc.vector.tensor_mul(wt[:, 5:9], wt[:, 5:9], g4)

    # ---- propagation iterations 1..PT-1 on folded planes
    for t in range(1, PT):
        Rt = RW - 2 * t
        xin, xout = xb[t % 2], xb[(t + 1) % 2]
        xoin = xoout = None
        if two_byte:
            xoin, xoout = xo[t % 2], xo[(t + 1) % 2]
        mul_A_split(w1, Rt, xin, 0)      # dil1 dw=0 (incl merged center)
        mm_sum(0, 3, Rt, True, False)
        mul_B(w2, Rt, xin, 3)
        mm_sum(3, 3, Rt, False, False)
        mul_C(w2, Rt, xin, 6)
        mm_sum(6, 3, Rt, False, False)
        mul_ODD(w1, Rt, xin, xoin, 0, -1)   # slots 0:3 free (PE consumed A)
        mm_sum(0, 3, Rt, False, False)
        mul_ODD(w1, Rt, xin, xoin, 3, +1)
        mm_sum(3, 3, Rt, False, False)
        mul_D(w2, Rt, xin, 6)
        if t == PT - 1 and Rt == HH:
            # last iteration: write the contiguous output tile directly and
            # skip the dead halo refresh (Rt != HH only under the PTE knob)
            mm_sum_last_fused(6, 2, Rt, lambda r0, rows: yc[:, r0:r0 + rows, :])
        else:
            mm_sum_last_fused(
                6, 2, Rt,
                lambda r0, rows: xout[:, 2 + r0:2 + r0 + rows, 2:2 + SW])
            halo_refresh(xout, xoout, Rt)

    # ---- store (yc was written directly by the last iteration's copy-out)
    if PT != g["PT"] or PT < 1:
        nc.vector.memset(yc, 0.0)  # PTE diagnostic knob: yc may be unwritten
    nc.sync.dma_start(out=y, in_=yc)
    ctx.close()


# ---------------------------------------------------------------- host side

_FLIPK = np.array([6, 7, 8, 3, 4, 5, 0, 1, 2])


def _prep_planes(a, half, g, np_dt):
    """a: [K, rows, W] slice -> [K, NS, rows, SW] strip layout (flip if half)."""
    K, rows, W = a.shape
    if half:
        a = a[:, ::-1]
    buf = np.zeros((K, rows, g["Wp"]), dtype=np_dt)
    buf[:, :, :W] = a
    return np.ascontiguousarray(
        buf.reshape(K, rows, g["NS"], g["SW"]).transpose(0, 2, 1, 3))


def host_shard(guided1, guided2, fuse, x, g):
    np_dt = np.dtype(g["dt_name"])
    NS, SW, SWH = g["NS"], g["SW"], g["SWH"]
    RW, RXL, RX, H, W, HH = g["RW"], g["RXL"], g["RX"], g["H"], g["W"], g["HH"]
    pl = np.eye(NS, k=1, dtype=np_dt)
    pr = np.eye(NS, k=-1, dtype=np_dt)
    pi = np.eye(NS, dtype=np_dt)
    cidx = (np.arange(NS) * SW)[:, None] + np.arange(SWH)[None, :]
    in_maps = []
    for c in range(2 * g["B"]):
        b, half = divmod(c, 2)
        wsl = slice(0, RW) if half == 0 else slice(H - RW, H)
        xsl = slice(0, RXL) if half == 0 else slice(H - RXL, H)
        g1p = _prep_planes(guided1[b][:, wsl], half, g, np_dt)
        g2p = _prep_planes(guided2[b][:, wsl], half, g, np_dt)
        if half:
            g1p, g2p = g1p[_FLIPK], g2p[_FLIPK]
        fzp = _prep_planes(fuse[b][:, wsl], half, g, np_dt)
        xa = x[b, 0][xsl]
        if half:
            xa = xa[::-1]
        xp = np.zeros((RX, g["Wp"] + 4), dtype=np_dt)
        xp[2:2 + RXL, 2:2 + W] = xa
        x0 = np.ascontiguousarray(xp[:, cidx].transpose(1, 0, 2))
        in_maps.append(dict(
            g1=np.ascontiguousarray(g1p), g2=np.ascontiguousarray(g2p),
            fz=np.ascontiguousarray(fzp), x0=x0, pl=pl, pr=pr, pi=pi))
    return in_maps


def host_gather(results, g):
    B, H, W, HH, NS, SW = g["B"], g["H"], g["W"], g["HH"], g["NS"], g["SW"]
    out = np.empty((B, 1, H, W), dtype=np.float32)
    for c, res in enumerate(results):
        b, half = divmod(c, 2)
        yimg = res["y"].astype(np.float32).transpose(1, 0, 2).reshape(
            HH, g["Wp"])[:, :W]
        if half:
            out[b, 0, HH:] = yimg[::-1]
        else:
            out[b, 0, :HH] = yimg
    return out


# ---------------------------------------------------------------- build+run

def build(g):
    import concourse.bacc as bacc
    import concourse.mybir as mybir
    import concourse.tile as tile

    DT = getattr(mybir.dt, g["dt_name"])
    F32 = mybir.dt.float32
    NS, SW, SWH, RW, RX, HH = (
        g["NS"], g["SW"], g["SWH"], g["RW"], g["RX"], g["HH"])
    nc = bacc.Bacc("TRN2", target_bir_lowering=False, debug=False,
                   num_devices=2 * g["B"])
    ins = dict(
        g1=nc.dram_tensor("g1", [9, NS, RW, SW], DT, kind="ExternalInput").ap(),
        g2=nc.dram_tensor("g2", [9, NS, RW, SW], DT, kind="ExternalInput").ap(),
        fz=nc.dram_tensor("fz", [2, NS, RW, SW], DT, kind="ExternalInput").ap(),
        x0=nc.dram_tensor("x0", [NS, RX, SWH], DT, kind="ExternalInput").ap(),
        pl=nc.dram_tensor("pl", [NS, NS], DT, kind="ExternalInput").ap(),
        pr=nc.dram_tensor("pr", [NS, NS], DT, kind="ExternalInput").ap(),
        pi=nc.dram_tensor("pi", [NS, NS], DT, kind="ExternalInput").ap(),
    )
    outs = dict(
        y=nc.dram_tensor("y", [NS, HH, SW], DT, kind="ExternalOutput").ap())
    with tile.TileContext(nc) as tc:
        emit(tc, outs, ins, g)
    nc.compile()
    return nc


_CACHE = {}


def _get_nc(g):
    key = tuple(sorted(g.items()))
    if key not in _CACHE:
        _CACHE[key] = build(g)
    return _CACHE[key]


def kernel(guided1, guided2, fuse, x, trace=False):
    from concourse.bass_utils import run_bass_kernel_spmd

    g = make_geom()
    nc = _get_nc(g)
    in_maps = host_shard(
        np.asarray(guided1, dtype=np.float32),
        np.asarray(guided2, dtype=np.float32),
        np.asarray(fuse, dtype=np.float32),
        np.asarray(x, dtype=np.float32), g)
    try:
        res = run_bass_kernel_spmd(nc, in_maps, list(range(2 * g["B"])),
                                   trace=trace)
    except (ImportError, ModuleNotFoundError):
        # NTFF profiling hook unavailable in this container; run untraced
        trace = False
        res = run_bass_kernel_spmd(nc, in_maps, list(range(2 * g["B"])),
                                   trace=False)
    out = host_gather(res.results, g)
    if trace:
        return out, res
    return out


def timeline_estimate_ns():
    """Cost-model (TimelineSim) estimate of per-core device exec time."""
    from concourse.timeline_sim import TimelineSim

    return TimelineSim(_get_nc(make_geom())).simulate()
